# revision 11
# baseline (speedup 1.0000x reference)
"""Trainium2 Bass kernel for nn_LocalDenseConv1D (unfold conv + BN(train) + PReLU).

Strategy: shard the 128 output positions (L) across 8 NeuronCores (16 each).
Host pre-transposes x [B,C,H,T] -> per-core fp16 slab [128, 17, 2048] whose
partition dim is (row-parity, channel) so one contiguous DMA stages the whole
input. The locally-connected contraction runs as 96 fp16 matmuls per core
(K=128 = 2 tap rows x 64 channels, M=128 = 2 output positions x 64
out-channels, N=512 (b,t) columns). Each output-position pair accumulates its
4 BT-chunks into 4 PSUM banks of one [128, 2048] tile, so a single ScalarE
activation evicts the pair (adding the conv bias). One bn_stats/bn_aggr pass
computes local BN stats, a tiny AllReduce sums per-channel (mean, E[x^2])
across cores, and a single fused ScalarE Prelu applies the BN affine + PReLU
over the whole [128, 16384] block before one strided DMA writes the fp16
output (upcast to fp32 on host).
"""
import numpy as np

import concourse.bass as bass
import concourse.tile as tile
from concourse import bacc, mybir
from concourse import bass_utils

F32 = mybir.dt.float32
F16 = mybir.dt.float16
AF = mybir.ActivationFunctionType

N_CORES = 8
B, C, H, T = 8, 64, 256, 256
O, L = 64, 128
BT = B * T                  # 2048 moving columns total
LC = L // N_CORES           # 16 output positions per core
PAIRS = LC // 2             # 8 pairs
SLAB = 2 * LC + 2           # 34 tap rows per core
NT = SLAB // 2              # 17 tap-pair tiles
CW = 512                    # matmul moving-dim chunk (PSUM bank)
NCH = BT // CW              # 4 chunks
BN_EPS = 1e-5

_CACHE = {}


def _build_nc(reps=1, timeline=False, prelu=True):
    nc = bacc.Bacc(
        "TRN2",
        target_bir_lowering=False,
        debug=False,
        enable_asserts=True,
        num_devices=1 if timeline else N_CORES,
    )
    xs = nc.dram_tensor("xs", [128, NT * BT], F16, kind="ExternalInput").ap()
    wb = nc.dram_tensor("wb", [128, 3 * PAIRS * 128], F16, kind="ExternalInput").ap()
    pp = nc.dram_tensor("pp", [128, 12], F32, kind="ExternalInput").ap()
    yo = nc.dram_tensor("yo", [LC, O, BT], F16, kind="ExternalOutput").ap()

    with tile.TileContext(nc) as tc:
        with (
            tc.tile_pool(name="xc", bufs=1) as xpool,
            tc.tile_pool(name="wp", bufs=1) as wpool,
            tc.tile_pool(name="yp", bufs=1) as ypool,
            tc.tile_pool(name="sp", bufs=1) as spool,
            tc.tile_pool(name="ps", bufs=2, space="PSUM") as psum,
            tc.tile_pool(name="dr", bufs=1, space="DRAM") as dram,
        ):
            for _rep in range(reps):
                wt = wpool.tile([128, 3 * PAIRS * 128], F16)
                nc.sync.dma_start(wt[:], wb[:])
                ppt = spool.tile([128, 12], F32)
                nc.sync.dma_start(ppt[:], pp[:])
                xt = xpool.tile([128, NT * BT], F16)
                nc.sync.dma_start(xt[:], xs[:])

                ysb = ypool.tile([128, PAIRS * BT], F32)
                for j in range(PAIRS):
                    pt = psum.tile([128, NCH * CW], F32, tag="acc")
                    for cc in range(NCH):
                        for k in range(3):
                            mm = j * 3 + k
                            nc.tensor.matmul(
                                pt[:, cc * CW : (cc + 1) * CW],
                                lhsT=wt[:, mm * 128 : (mm + 1) * 128],
                                rhs=xt[
                                    :,
                                    (2 * j + k) * BT + cc * CW :
                                    (2 * j + k) * BT + (cc + 1) * CW,
                                ],
                                start=(k == 0),
                                stop=(k == 2),
                            )
                    # evict whole pair (4 banks) + conv bias in one activation
                    nc.scalar.activation(
                        ysb[:, j * BT : (j + 1) * BT],
                        pt[:],
                        AF.Identity,
                        bias=ppt[:, j : j + 1],
                        scale=1.0,
                    )

                # local (sum, sum-of-squares) per partition in 2 instructions:
                # ScalarE accumulator reduces the whole [128, 16384] block.
                obuf = ypool.tile([128, PAIRS * BT], F16)
                agin = spool.tile([128, 2], F32)
                nc.scalar.activation(
                    obuf[:], ysb[:], AF.Identity, accum_out=agin[:, 0:1]
                )
                nc.scalar.activation(
                    obuf[:], ysb[:], AF.Square, accum_out=agin[:, 1:2]
                )

                agi = dram.tile([128, 2], F32)
                ago = dram.tile([128, 2], F32)
                nc.sync.dma_start(agi[:], agin[:])
                if timeline:
                    nc.sync.dma_start(ago[:], agi[:])
                else:
                    nc.gpsimd.collective_compute(
                        "AllReduce",
                        mybir.AluOpType.add,
                        replica_groups=[list(range(N_CORES))],
                        ins=[agi.opt()],
                        outs=[ago.opt()],
                    )
                g = spool.tile([128, 2], F32)
                nc.sync.dma_start(g[:], ago[:])

                # combine the two partition halves (lp=0 / lp=1) per channel o
                tmp = spool.tile([128, 2], F32)
                nc.sync.dma_start(tmp[0:64, :], g[64:128, :])
                s2 = spool.tile([128, 2], F32)
                nc.vector.tensor_add(s2[0:64, :], g[0:64, :], tmp[0:64, :])
                mm2 = spool.tile([128, 2], F32)
                nc.scalar.mul(
                    mm2[0:64, :], s2[0:64, :], 1.0 / (2 * N_CORES * PAIRS * BT)
                )

                # scale = gamma * rsqrt(var+eps); shift = beta - mean*scale
                var = spool.tile([128, 1], F32)
                nc.vector.tensor_mul(var[0:64], mm2[0:64, 0:1], mm2[0:64, 0:1])
                nc.vector.tensor_sub(var[0:64], mm2[0:64, 1:2], var[0:64])
                vae = spool.tile([128, 1], F32)
                nc.vector.tensor_scalar_add(vae[0:64], var[0:64], BN_EPS)
                inv = spool.tile([128, 1], F32)
                nc.vector.reciprocal(inv[0:64], vae[0:64])
                st = spool.tile([128, 1], F32)
                nc.scalar.sqrt(st[0:64], inv[0:64])
                sclsht = spool.tile([128, 2], F32)
                nc.vector.tensor_mul(sclsht[0:64, 0:1], st[0:64], ppt[0:64, 8:9])
                nc.vector.tensor_mul(sclsht[0:64, 1:2], mm2[0:64, 0:1], sclsht[0:64, 0:1])
                nc.vector.tensor_sub(sclsht[0:64, 1:2], ppt[0:64, 9:10], sclsht[0:64, 1:2])
                nc.sync.dma_start(sclsht[64:128, :], sclsht[0:64, :])

                # fused BN affine + PReLU over the whole block, fp16 out
                nc.scalar.activation(
                    obuf[:],
                    ysb[:],
                    AF.Prelu if prelu else AF.Identity,
                    bias=sclsht[:, 1:2],
                    scale=sclsht[:, 0:1],
                    alpha=ppt[:, 10:11],
                )
                nc.sync.dma_start(
                    yo.rearrange("(j lp) o n -> (lp o) j n", lp=2),
                    obuf[:].rearrange("p (j n) -> p j n", n=BT),
                )
    nc.compile()
    return nc


def _get_nc():
    if "nc" not in _CACHE:
        _CACHE["nc"] = _build_nc()
    return _CACHE["nc"]


def _prep_in_maps(x, weight, bias, gamma, beta, prelu_a):
    x = np.ascontiguousarray(x, dtype=np.float32)
    weight = np.asarray(weight, dtype=np.float32)
    bias = np.asarray(bias, dtype=np.float32)
    gamma = np.asarray(gamma, dtype=np.float32)
    beta = np.asarray(beta, dtype=np.float32)
    prelu_a = np.float32(np.asarray(prelu_a))

    # padded tap-row-major input: xtp[j] = x[:, :, j-1, :] as [C, B*T], fp16
    xtp = np.zeros((H + 2, C, B, T), np.float16)
    xtp[1 : H + 1] = np.transpose(x, (2, 1, 0, 3))
    xtp = xtp.reshape(H + 2, C, BT)

    wv = weight.reshape(C, 3, O, L)  # [c, kh, o, l]
    lidx = np.arange(L).reshape(N_CORES, PAIRS, 2)
    lA, lB = lidx[:, :, 0], lidx[:, :, 1]

    def pick(kh, l2):  # -> [core, j, c, o]
        return np.transpose(wv[:, kh][:, :, l2], (2, 3, 0, 1))

    wball = np.zeros((N_CORES, PAIRS, 3, 2, C, 2, O), np.float32)
    wball[:, :, 0, 0, :, 0, :] = pick(0, lA)
    wball[:, :, 0, 1, :, 0, :] = pick(1, lA)
    wball[:, :, 1, 0, :, 0, :] = pick(2, lA)
    wball[:, :, 1, 0, :, 1, :] = pick(0, lB)
    wball[:, :, 1, 1, :, 1, :] = pick(1, lB)
    wball[:, :, 2, 0, :, 1, :] = pick(2, lB)
    # [core, mm=3*PAIRS, K=128, M=128] -> partition-major [core, K, mm, M]
    wball = wball.reshape(N_CORES, 3 * PAIRS, 128, 128)
    wball = np.ascontiguousarray(
        np.transpose(wball, (0, 2, 1, 3)), dtype=np.float16
    ).reshape(N_CORES, 128, 3 * PAIRS * 128)

    bv = bias.reshape(O, N_CORES, PAIRS, 2)  # [o, core, j, lp]
    cball = np.ascontiguousarray(
        np.transpose(bv, (1, 3, 0, 2)).reshape(N_CORES, 128, PAIRS)
    )

    in_maps = []
    for i in range(N_CORES):
        slab = xtp[32 * i : 32 * i + SLAB]                # [34, C, BT]
        slab = slab.reshape(NT, 2, C, BT)                 # [t, jpar, c, n]
        slab = np.ascontiguousarray(np.transpose(slab, (1, 2, 0, 3)))
        pp = np.zeros((128, 12), np.float32)
        pp[:, :PAIRS] = cball[i]
        pp[:, 8] = np.concatenate([gamma, gamma])
        pp[:, 9] = np.concatenate([beta, beta])
        pp[:, 10] = prelu_a
        in_maps.append(
            {
                "xs": slab.reshape(128, NT * BT),
                "wb": wball[i],
                "pp": pp,
            }
        )
    return in_maps


def _unshard(results):
    outs = [
        results[i]["yo"].reshape(LC, O, B, T).transpose(2, 1, 0, 3)
        for i in range(N_CORES)
    ]
    return np.ascontiguousarray(
        np.concatenate(outs, axis=2), dtype=np.float32
    )


def kernel(x, weight, bias, gamma, beta, prelu_a):
    nc = _get_nc()
    in_maps = _prep_in_maps(x, weight, bias, gamma, beta, prelu_a)
    res = bass_utils.run_bass_kernel_spmd(
        nc, in_maps, core_ids=list(range(N_CORES)), trace=False
    )
    return _unshard(res.results)


# revision 21
# speedup vs baseline: 1.2280x; 1.2280x over previous
"""Trainium2 Bass kernel for nn_LocalDenseConv1D (unfold conv + BN(train) + PReLU).

Strategy: shard the 128 output positions (L) across 8 NeuronCores (16 each).
Host pre-transposes x [B,C,H,T] -> per-core fp16 slab [128, 17, 2048] whose
partition dim is (row-parity, channel) so one contiguous DMA stages the whole
input. The locally-connected contraction runs as 96 fp16 matmuls per core
(K=128 = 2 tap rows x 64 channels, M=128 = 2 output positions x 64
out-channels, N=512 (b,t) columns). Each output-position pair accumulates its
4 BT-chunks into 4 PSUM banks of one [128, 2048] tile, so a single ScalarE
activation evicts the pair (adding the conv bias). One bn_stats/bn_aggr pass
computes local BN stats, a tiny AllReduce sums per-channel (mean, E[x^2])
across cores, and a single fused ScalarE Prelu applies the BN affine + PReLU
over the whole [128, 16384] block before one strided DMA writes the fp16
output (upcast to fp32 on host).
"""
import numpy as np

import concourse.bass as bass
import concourse.tile as tile
from concourse import bacc, mybir
from concourse import bass_utils

F32 = mybir.dt.float32
F16 = mybir.dt.float16
AF = mybir.ActivationFunctionType

N_CORES = 8
B, C, H, T = 8, 64, 256, 256
O, L = 64, 128
BT = B * T                  # 2048 moving columns total
LC = L // N_CORES           # 16 output positions per core
PAIRS = LC // 2             # 8 pairs
SLAB = 2 * LC + 2           # 34 tap rows per core
NT = SLAB // 2              # 17 tap-pair tiles
CW = 512                    # matmul moving-dim chunk (PSUM bank)
NCH = BT // CW              # 4 chunks
BN_EPS = 1e-5

_CACHE = {}


def _build_nc(reps=1, timeline=False, prelu=True):
    nc = bacc.Bacc(
        "TRN2",
        target_bir_lowering=False,
        debug=False,
        enable_asserts=True,
        num_devices=1 if timeline else N_CORES,
    )
    xs = nc.dram_tensor("xs", [128, NT * BT], F16, kind="ExternalInput").ap()
    wb = nc.dram_tensor("wb", [128, 3 * PAIRS * 128], F16, kind="ExternalInput").ap()
    pp = nc.dram_tensor("pp", [128, 12], F32, kind="ExternalInput").ap()
    yo = nc.dram_tensor("yo", [LC, O, BT], F16, kind="ExternalOutput").ap()

    with tile.TileContext(nc) as tc:
        with (
            tc.tile_pool(name="xc", bufs=1) as xpool,
            tc.tile_pool(name="wp", bufs=1) as wpool,
            tc.tile_pool(name="yp", bufs=1) as ypool,
            tc.tile_pool(name="sp", bufs=1) as spool,
            tc.tile_pool(name="ps", bufs=2, space="PSUM") as psum,
            tc.tile_pool(name="dr", bufs=1, space="DRAM") as dram,
        ):
            for _rep in range(reps):
                wt = wpool.tile([128, 3 * PAIRS * 128], F16)
                nc.sync.dma_start(wt[:], wb[:])
                ppt = spool.tile([128, 12], F32)
                nc.sync.dma_start(ppt[:], pp[:])
                # stage the input slab in two halves so the first pairs'
                # matmuls overlap the second half's DMA
                xt = xpool.tile([128, NT * BT], F16)
                nc.sync.dma_start(xt[:, : 9 * BT], xs[:, : 9 * BT])
                nc.sync.dma_start(xt[:, 9 * BT :], xs[:, 9 * BT :])

                ysb = ypool.tile([128, PAIRS * BT], F32)
                sumc = spool.tile([128, PAIRS], F32)
                for j in range(PAIRS):
                    pt = psum.tile([128, NCH * CW], F32, tag="acc")
                    for cc in range(NCH):
                        for k in range(3):
                            mm = j * 3 + k
                            nc.tensor.matmul(
                                pt[:, cc * CW : (cc + 1) * CW],
                                lhsT=wt[:, mm * 128 : (mm + 1) * 128],
                                rhs=xt[
                                    :,
                                    (2 * j + k) * BT + cc * CW :
                                    (2 * j + k) * BT + (cc + 1) * CW,
                                ],
                                start=(k == 0),
                                stop=(k == 2),
                            )
                    # evict whole pair (4 banks) + conv bias in one activation;
                    # the ScalarE accumulator gives this pair's sum for free
                    nc.scalar.activation(
                        ysb[:, j * BT : (j + 1) * BT],
                        pt[:],
                        AF.Identity,
                        bias=ppt[:, j : j + 1],
                        scale=1.0,
                        accum_out=sumc[:, j : j + 1],
                    )

                # per-partition (sum, sum-of-squares) for the cross-core sum
                obuf = ypool.tile([128, PAIRS * BT], F16)
                agin = spool.tile([128, 2], F32)
                nc.vector.tensor_reduce(
                    agin[:, 0:1],
                    sumc[:],
                    axis=mybir.AxisListType.X,
                    op=mybir.AluOpType.add,
                )
                nc.scalar.activation(
                    obuf[:], ysb[:], AF.Square, accum_out=agin[:, 1:2]
                )

                agi = dram.tile([128, 2], F32)
                ago = dram.tile([128, 2], F32)
                nc.sync.dma_start(agi[:], agin[:])
                if timeline:
                    nc.sync.dma_start(ago[:], agi[:])
                else:
                    nc.gpsimd.collective_compute(
                        "AllReduce",
                        mybir.AluOpType.add,
                        replica_groups=[list(range(N_CORES))],
                        ins=[agi.opt()],
                        outs=[ago.opt()],
                    )
                g = spool.tile([128, 2], F32)
                nc.sync.dma_start(g[:], ago[:])

                # combine the two partition halves (lp=0 / lp=1) per channel o
                tmp = spool.tile([128, 2], F32)
                nc.sync.dma_start(tmp[0:64, :], g[64:128, :])
                s2 = spool.tile([128, 2], F32)
                nc.vector.tensor_add(s2[0:64, :], g[0:64, :], tmp[0:64, :])
                mm2 = spool.tile([128, 2], F32)
                nc.scalar.mul(
                    mm2[0:64, :], s2[0:64, :], 1.0 / (2 * N_CORES * PAIRS * BT)
                )

                # scale = gamma * rsqrt(var+eps); shift = beta - mean*scale
                var = spool.tile([128, 1], F32)
                nc.vector.tensor_mul(var[0:64], mm2[0:64, 0:1], mm2[0:64, 0:1])
                nc.vector.tensor_sub(var[0:64], mm2[0:64, 1:2], var[0:64])
                vae = spool.tile([128, 1], F32)
                nc.vector.tensor_scalar_add(vae[0:64], var[0:64], BN_EPS)
                inv = spool.tile([128, 1], F32)
                nc.vector.reciprocal(inv[0:64], vae[0:64])
                st = spool.tile([128, 1], F32)
                nc.scalar.sqrt(st[0:64], inv[0:64])
                sclsht = spool.tile([128, 2], F32)
                nc.vector.tensor_mul(sclsht[0:64, 0:1], st[0:64], ppt[0:64, 8:9])
                nc.vector.tensor_mul(sclsht[0:64, 1:2], mm2[0:64, 0:1], sclsht[0:64, 0:1])
                nc.vector.tensor_sub(sclsht[0:64, 1:2], ppt[0:64, 9:10], sclsht[0:64, 1:2])
                nc.sync.dma_start(sclsht[64:128, :], sclsht[0:64, :])

                # fused BN affine + PReLU over the whole block, fp16 out
                nc.scalar.activation(
                    obuf[:],
                    ysb[:],
                    AF.Prelu if prelu else AF.Identity,
                    bias=sclsht[:, 1:2],
                    scale=sclsht[:, 0:1],
                    alpha=ppt[:, 10:11],
                )
                nc.sync.dma_start(
                    yo.rearrange("(j lp) o n -> (lp o) j n", lp=2),
                    obuf[:].rearrange("p (j n) -> p j n", n=BT),
                )
    nc.compile()
    return nc


def _get_nc():
    if "nc" not in _CACHE:
        _CACHE["nc"] = _build_nc()
    return _CACHE["nc"]


def _prep_in_maps(x, weight, bias, gamma, beta, prelu_a):
    x = np.ascontiguousarray(x, dtype=np.float32)
    weight = np.asarray(weight, dtype=np.float32)
    bias = np.asarray(bias, dtype=np.float32)
    gamma = np.asarray(gamma, dtype=np.float32)
    beta = np.asarray(beta, dtype=np.float32)
    prelu_a = np.float32(np.asarray(prelu_a))

    # padded tap-row-major input: xtp[j] = x[:, :, j-1, :] as [C, B*T], fp16
    xtp = np.zeros((H + 2, C, B, T), np.float16)
    xtp[1 : H + 1] = np.transpose(x, (2, 1, 0, 3))
    xtp = xtp.reshape(H + 2, C, BT)

    wv = weight.reshape(C, 3, O, L)  # [c, kh, o, l]
    lidx = np.arange(L).reshape(N_CORES, PAIRS, 2)
    lA, lB = lidx[:, :, 0], lidx[:, :, 1]

    def pick(kh, l2):  # -> [core, j, c, o]
        return np.transpose(wv[:, kh][:, :, l2], (2, 3, 0, 1))

    wball = np.zeros((N_CORES, PAIRS, 3, 2, C, 2, O), np.float32)
    wball[:, :, 0, 0, :, 0, :] = pick(0, lA)
    wball[:, :, 0, 1, :, 0, :] = pick(1, lA)
    wball[:, :, 1, 0, :, 0, :] = pick(2, lA)
    wball[:, :, 1, 0, :, 1, :] = pick(0, lB)
    wball[:, :, 1, 1, :, 1, :] = pick(1, lB)
    wball[:, :, 2, 0, :, 1, :] = pick(2, lB)
    # [core, mm=3*PAIRS, K=128, M=128] -> partition-major [core, K, mm, M]
    wball = wball.reshape(N_CORES, 3 * PAIRS, 128, 128)
    wball = np.ascontiguousarray(
        np.transpose(wball, (0, 2, 1, 3)), dtype=np.float16
    ).reshape(N_CORES, 128, 3 * PAIRS * 128)

    bv = bias.reshape(O, N_CORES, PAIRS, 2)  # [o, core, j, lp]
    cball = np.ascontiguousarray(
        np.transpose(bv, (1, 3, 0, 2)).reshape(N_CORES, 128, PAIRS)
    )

    in_maps = []
    for i in range(N_CORES):
        slab = xtp[32 * i : 32 * i + SLAB]                # [34, C, BT]
        slab = slab.reshape(NT, 2, C, BT)                 # [t, jpar, c, n]
        slab = np.ascontiguousarray(np.transpose(slab, (1, 2, 0, 3)))
        pp = np.zeros((128, 12), np.float32)
        pp[:, :PAIRS] = cball[i]
        pp[:, 8] = np.concatenate([gamma, gamma])
        pp[:, 9] = np.concatenate([beta, beta])
        pp[:, 10] = prelu_a
        in_maps.append(
            {
                "xs": slab.reshape(128, NT * BT),
                "wb": wball[i],
                "pp": pp,
            }
        )
    return in_maps


def _unshard(results):
    outs = [
        results[i]["yo"].reshape(LC, O, B, T).transpose(2, 1, 0, 3)
        for i in range(N_CORES)
    ]
    return np.ascontiguousarray(
        np.concatenate(outs, axis=2), dtype=np.float32
    )


def kernel(x, weight, bias, gamma, beta, prelu_a):
    nc = _get_nc()
    in_maps = _prep_in_maps(x, weight, bias, gamma, beta, prelu_a)
    res = bass_utils.run_bass_kernel_spmd(
        nc, in_maps, core_ids=list(range(N_CORES)), trace=False
    )
    return _unshard(res.results)


# revision 22
# speedup vs baseline: 1.3980x; 1.1384x over previous
"""Trainium2 Bass kernel for nn_LocalDenseConv1D (unfold conv + BN(train) + PReLU).

Strategy: shard the 128 output positions (L) across 8 NeuronCores (16 each).
Host pre-transposes x [B,C,H,T] -> per-core fp16 slab [128, 17, 2048] whose
partition dim is (row-parity, channel) so one contiguous DMA stages the whole
input. The locally-connected contraction runs as 96 fp16 matmuls per core
(K=128 = 2 tap rows x 64 channels, M=128 = 2 output positions x 64
out-channels, N=512 (b,t) columns). Each output-position pair accumulates its
4 BT-chunks into 4 PSUM banks of one [128, 2048] tile, so a single ScalarE
activation evicts the pair (adding the conv bias) while its accumulator
emits the pair's running sum; one ScalarE Square pass emits sum-of-squares.
A tiny AllReduce sums per-channel (sum, sum-sq) across cores, and a single
fused ScalarE Prelu applies the BN affine + PReLU over the whole
[128, 16384] block before one strided DMA writes the fp16 output (upcast to
fp32 on host).
"""
import numpy as np

import concourse.bass as bass
import concourse.tile as tile
from concourse import bacc, mybir
from concourse import bass_utils

F32 = mybir.dt.float32
F16 = mybir.dt.float16
AF = mybir.ActivationFunctionType

N_CORES = 8
B, C, H, T = 8, 64, 256, 256
O, L = 64, 128
BT = B * T                  # 2048 moving columns total
LC = L // N_CORES           # 16 output positions per core
PAIRS = LC // 2             # 8 pairs
SLAB = 2 * LC + 2           # 34 tap rows per core
NT = SLAB // 2              # 17 tap-pair tiles
CW = 512                    # matmul moving-dim chunk (PSUM bank)
NCH = BT // CW              # 4 chunks
BN_EPS = 1e-5

_CACHE = {}


def _build_nc(reps=1, timeline=False, prelu=True):
    nc = bacc.Bacc(
        "TRN2",
        target_bir_lowering=False,
        debug=False,
        enable_asserts=True,
        num_devices=1 if timeline else N_CORES,
    )
    xs = nc.dram_tensor("xs", [128, NT * BT], F16, kind="ExternalInput").ap()
    wb = nc.dram_tensor("wb", [128, 3 * PAIRS * 128], F16, kind="ExternalInput").ap()
    pp = nc.dram_tensor("pp", [128, 12], F32, kind="ExternalInput").ap()
    yo = nc.dram_tensor("yo", [LC, O, BT], F16, kind="ExternalOutput").ap()

    with tile.TileContext(nc) as tc:
        with (
            tc.tile_pool(name="xc", bufs=1) as xpool,
            tc.tile_pool(name="wp", bufs=1) as wpool,
            tc.tile_pool(name="yp", bufs=1) as ypool,
            tc.tile_pool(name="sp", bufs=1) as spool,
            tc.tile_pool(name="ps", bufs=2, space="PSUM") as psum,
            tc.tile_pool(name="dr", bufs=1, space="DRAM") as dram,
        ):
            for _rep in range(reps):
                wt = wpool.tile([128, 3 * PAIRS * 128], F16)
                nc.sync.dma_start(wt[:], wb[:])
                ppt = spool.tile([128, 12], F32)
                nc.sync.dma_start(ppt[:], pp[:])
                # stage the input slab in three slices so early pairs'
                # matmuls overlap the remaining DMA (pair j needs tap
                # tiles 2j..2j+2: slice1 covers pairs 0-1, slice2 2-4)
                xt = xpool.tile([128, NT * BT], F16)
                nc.sync.dma_start(xt[:, : 5 * BT], xs[:, : 5 * BT])
                nc.sync.dma_start(xt[:, 5 * BT : 11 * BT], xs[:, 5 * BT : 11 * BT])
                nc.sync.dma_start(xt[:, 11 * BT :], xs[:, 11 * BT :])

                ysb = ypool.tile([128, PAIRS * BT], F32)
                sumc = spool.tile([128, PAIRS], F32)
                for j in range(PAIRS):
                    pt = psum.tile([128, NCH * CW], F32, tag="acc")
                    for cc in range(NCH):
                        for k in range(3):
                            mm = j * 3 + k
                            nc.tensor.matmul(
                                pt[:, cc * CW : (cc + 1) * CW],
                                lhsT=wt[:, mm * 128 : (mm + 1) * 128],
                                rhs=xt[
                                    :,
                                    (2 * j + k) * BT + cc * CW :
                                    (2 * j + k) * BT + (cc + 1) * CW,
                                ],
                                start=(k == 0),
                                stop=(k == 2),
                            )
                    # evict whole pair (4 banks) + conv bias in one activation;
                    # the ScalarE accumulator gives this pair's sum for free
                    nc.scalar.activation(
                        ysb[:, j * BT : (j + 1) * BT],
                        pt[:],
                        AF.Identity,
                        bias=ppt[:, j : j + 1],
                        scale=1.0,
                        accum_out=sumc[:, j : j + 1],
                    )

                # per-partition (sum, sum-of-squares) for the cross-core sum
                obuf = ypool.tile([128, PAIRS * BT], F16)
                agin = spool.tile([128, 2], F32)
                nc.vector.tensor_reduce(
                    agin[:, 0:1],
                    sumc[:],
                    axis=mybir.AxisListType.X,
                    op=mybir.AluOpType.add,
                )
                nc.scalar.activation(
                    obuf[:], ysb[:], AF.Square, accum_out=agin[:, 1:2]
                )

                agi = dram.tile([128, 2], F32)
                ago = dram.tile([128, 2], F32)
                nc.sync.dma_start(agi[:], agin[:])
                if timeline:
                    nc.sync.dma_start(ago[:], agi[:])
                else:
                    nc.gpsimd.collective_compute(
                        "AllReduce",
                        mybir.AluOpType.add,
                        replica_groups=[list(range(N_CORES))],
                        ins=[agi.opt()],
                        outs=[ago.opt()],
                    )
                # load both partition halves (lp=0/1) side by side on
                # partitions 0-63, so no cross-half shuffle is needed
                g = spool.tile([128, 4], F32)
                nc.sync.dma_start(
                    g[0:64, :].rearrange("p (h v) -> p h v", h=2),
                    ago.rearrange("(h p) v -> p h v", h=2),
                )
                s2 = spool.tile([128, 2], F32)
                nc.vector.tensor_add(s2[0:64, :], g[0:64, 0:2], g[0:64, 2:4])
                mm2 = spool.tile([128, 2], F32)
                nc.scalar.mul(
                    mm2[0:64, :], s2[0:64, :], 1.0 / (2 * N_CORES * PAIRS * BT)
                )

                # scale = gamma * rsqrt(var+eps); shift = beta - mean*scale
                var = spool.tile([128, 1], F32)
                nc.vector.tensor_mul(var[0:64], mm2[0:64, 0:1], mm2[0:64, 0:1])
                nc.vector.tensor_sub(var[0:64], mm2[0:64, 1:2], var[0:64])
                sdt = spool.tile([128, 1], F32)
                nc.scalar.activation(
                    sdt[0:64], var[0:64], AF.Sqrt, bias=ppt[0:64, 11:12]
                )
                st = spool.tile([128, 1], F32)
                nc.vector.reciprocal(st[0:64], sdt[0:64])
                sclsht = spool.tile([128, 2], F32)
                nc.vector.tensor_mul(sclsht[0:64, 0:1], st[0:64], ppt[0:64, 8:9])
                nc.vector.tensor_mul(sclsht[0:64, 1:2], mm2[0:64, 0:1], sclsht[0:64, 0:1])
                nc.vector.tensor_sub(sclsht[0:64, 1:2], ppt[0:64, 9:10], sclsht[0:64, 1:2])
                nc.sync.dma_start(sclsht[64:128, :], sclsht[0:64, :])

                # fused BN affine + PReLU, split in halves so the first
                # half's output DMA overlaps the second half's activation
                yov = yo.rearrange("(j lp) o n -> (lp o) j n", lp=2)
                HB = PAIRS * BT // 2
                for h in range(2):
                    nc.scalar.activation(
                        obuf[:, h * HB : (h + 1) * HB],
                        ysb[:, h * HB : (h + 1) * HB],
                        AF.Prelu if prelu else AF.Identity,
                        bias=sclsht[:, 1:2],
                        scale=sclsht[:, 0:1],
                        alpha=ppt[:, 10:11],
                    )
                    nc.sync.dma_start(
                        yov[:, h * 4 : (h + 1) * 4, :],
                        obuf[:, h * HB : (h + 1) * HB].rearrange(
                            "p (j n) -> p j n", n=BT
                        ),
                    )
    nc.compile()
    return nc


def _get_nc():
    if "nc" not in _CACHE:
        _CACHE["nc"] = _build_nc()
    return _CACHE["nc"]


def _prep_in_maps(x, weight, bias, gamma, beta, prelu_a):
    x = np.ascontiguousarray(x, dtype=np.float32)
    weight = np.asarray(weight, dtype=np.float32)
    bias = np.asarray(bias, dtype=np.float32)
    gamma = np.asarray(gamma, dtype=np.float32)
    beta = np.asarray(beta, dtype=np.float32)
    prelu_a = np.float32(np.asarray(prelu_a))

    # padded tap-row-major input: xtp[j] = x[:, :, j-1, :] as [C, B*T], fp16
    xtp = np.zeros((H + 2, C, B, T), np.float16)
    xtp[1 : H + 1] = np.transpose(x, (2, 1, 0, 3))
    xtp = xtp.reshape(H + 2, C, BT)

    wv = weight.reshape(C, 3, O, L)  # [c, kh, o, l]
    lidx = np.arange(L).reshape(N_CORES, PAIRS, 2)
    lA, lB = lidx[:, :, 0], lidx[:, :, 1]

    def pick(kh, l2):  # -> [core, j, c, o]
        return np.transpose(wv[:, kh][:, :, l2], (2, 3, 0, 1))

    wball = np.zeros((N_CORES, PAIRS, 3, 2, C, 2, O), np.float32)
    wball[:, :, 0, 0, :, 0, :] = pick(0, lA)
    wball[:, :, 0, 1, :, 0, :] = pick(1, lA)
    wball[:, :, 1, 0, :, 0, :] = pick(2, lA)
    wball[:, :, 1, 0, :, 1, :] = pick(0, lB)
    wball[:, :, 1, 1, :, 1, :] = pick(1, lB)
    wball[:, :, 2, 0, :, 1, :] = pick(2, lB)
    # [core, mm=3*PAIRS, K=128, M=128] -> partition-major [core, K, mm, M]
    wball = wball.reshape(N_CORES, 3 * PAIRS, 128, 128)
    wball = np.ascontiguousarray(
        np.transpose(wball, (0, 2, 1, 3)), dtype=np.float16
    ).reshape(N_CORES, 128, 3 * PAIRS * 128)

    bv = bias.reshape(O, N_CORES, PAIRS, 2)  # [o, core, j, lp]
    cball = np.ascontiguousarray(
        np.transpose(bv, (1, 3, 0, 2)).reshape(N_CORES, 128, PAIRS)
    )

    in_maps = []
    for i in range(N_CORES):
        slab = xtp[32 * i : 32 * i + SLAB]                # [34, C, BT]
        slab = slab.reshape(NT, 2, C, BT)                 # [t, jpar, c, n]
        slab = np.ascontiguousarray(np.transpose(slab, (1, 2, 0, 3)))
        pp = np.zeros((128, 12), np.float32)
        pp[:, :PAIRS] = cball[i]
        pp[:, 8] = np.concatenate([gamma, gamma])
        pp[:, 9] = np.concatenate([beta, beta])
        pp[:, 10] = prelu_a
        pp[:, 11] = BN_EPS
        in_maps.append(
            {
                "xs": slab.reshape(128, NT * BT),
                "wb": wball[i],
                "pp": pp,
            }
        )
    return in_maps


def _unshard(results):
    outs = [
        results[i]["yo"].reshape(LC, O, B, T).transpose(2, 1, 0, 3)
        for i in range(N_CORES)
    ]
    return np.ascontiguousarray(
        np.concatenate(outs, axis=2), dtype=np.float32
    )


def kernel(x, weight, bias, gamma, beta, prelu_a):
    nc = _get_nc()
    in_maps = _prep_in_maps(x, weight, bias, gamma, beta, prelu_a)
    res = bass_utils.run_bass_kernel_spmd(
        nc, in_maps, core_ids=list(range(N_CORES)), trace=False
    )
    return _unshard(res.results)


# revision 25
# speedup vs baseline: 1.6374x; 1.1712x over previous
"""Trainium2 Bass kernel for nn_LocalDenseConv1D (unfold conv + BN(train) + PReLU).

Strategy: shard the 128 output positions (L) across 8 NeuronCores (16 each).
Host pre-transposes x [B,C,H,T] -> per-core fp16 slab [128, 17, 2048] whose
partition dim is (row-parity, channel) so one contiguous DMA stages the whole
input. The locally-connected contraction runs as 96 fp16 matmuls per core
(K=128 = 2 tap rows x 64 channels, M=128 = 2 output positions x 64
out-channels, N=512 (b,t) columns). Each output-position pair accumulates its
4 BT-chunks into 4 PSUM banks of one [128, 2048] tile, so a single ScalarE
activation evicts the pair (adding the conv bias) while its accumulator
emits the pair's running sum; one ScalarE Square pass emits sum-of-squares.
A tiny AllReduce sums per-channel (sum, sum-sq) across cores, and a single
fused ScalarE Prelu applies the BN affine + PReLU over the whole
[128, 16384] block before one strided DMA writes the fp16 output (upcast to
fp32 on host).
"""
import numpy as np

import concourse.bass as bass
import concourse.tile as tile
from concourse import bacc, mybir
from concourse import bass_utils

F32 = mybir.dt.float32
F16 = mybir.dt.float16
AF = mybir.ActivationFunctionType

N_CORES = 8
B, C, H, T = 8, 64, 256, 256
O, L = 64, 128
BT = B * T                  # 2048 moving columns total
LC = L // N_CORES           # 16 output positions per core
PAIRS = LC // 2             # 8 pairs
SLAB = 2 * LC + 2           # 34 tap rows per core
NT = SLAB // 2              # 17 tap-pair tiles
CW = 512                    # matmul moving-dim chunk (PSUM bank)
NCH = BT // CW              # 4 chunks
BN_EPS = 1e-5

_CACHE = {}


def _build_nc(reps=1, timeline=False, prelu=True):
    nc = bacc.Bacc(
        "TRN2",
        target_bir_lowering=False,
        debug=False,
        enable_asserts=True,
        num_devices=1 if timeline else N_CORES,
    )
    xs = nc.dram_tensor("xs", [128, NT * BT], F16, kind="ExternalInput").ap()
    wb = nc.dram_tensor("wb", [128, 3 * PAIRS * 128], F16, kind="ExternalInput").ap()
    pp = nc.dram_tensor("pp", [128, 12], F32, kind="ExternalInput").ap()
    yo = nc.dram_tensor("yo", [LC, O, BT], F16, kind="ExternalOutput").ap()

    with tile.TileContext(nc) as tc:
        with (
            tc.tile_pool(name="xc", bufs=1) as xpool,
            tc.tile_pool(name="wp", bufs=1) as wpool,
            tc.tile_pool(name="yp", bufs=1) as ypool,
            tc.tile_pool(name="sp", bufs=1) as spool,
            tc.tile_pool(name="ps", bufs=2, space="PSUM") as psum,
            tc.tile_pool(name="dr", bufs=1, space="DRAM") as dram,
        ):
            for _rep in range(reps):
                wt = wpool.tile([128, 3 * PAIRS * 128], F16)
                nc.sync.dma_start(wt[:], wb[:])
                ppt = spool.tile([128, 12], F32)
                nc.sync.dma_start(ppt[:], pp[:])
                # stage the input slab one tap-tile (512KB) per DMA: pair j
                # only waits for tiles 2j..2j+2, so matmuls overlap the
                # remaining input DMA maximally
                xt = xpool.tile([128, NT * BT], F16)
                for tti in range(NT):
                    nc.sync.dma_start(
                        xt[:, tti * BT : (tti + 1) * BT],
                        xs[:, tti * BT : (tti + 1) * BT],
                    )

                ysb = ypool.tile([128, PAIRS * BT], F32)
                sumc = spool.tile([128, PAIRS], F32)
                for j in range(PAIRS):
                    pt = psum.tile([128, NCH * CW], F32, tag="acc")
                    for cc in range(NCH):
                        for k in range(3):
                            mm = j * 3 + k
                            nc.tensor.matmul(
                                pt[:, cc * CW : (cc + 1) * CW],
                                lhsT=wt[:, mm * 128 : (mm + 1) * 128],
                                rhs=xt[
                                    :,
                                    (2 * j + k) * BT + cc * CW :
                                    (2 * j + k) * BT + (cc + 1) * CW,
                                ],
                                start=(k == 0),
                                stop=(k == 2),
                            )
                    # evict whole pair (4 banks) + conv bias in one activation;
                    # the ScalarE accumulator gives this pair's sum for free
                    nc.scalar.activation(
                        ysb[:, j * BT : (j + 1) * BT],
                        pt[:],
                        AF.Identity,
                        bias=ppt[:, j : j + 1],
                        scale=1.0,
                        accum_out=sumc[:, j : j + 1],
                    )

                # per-partition (sum, sum-of-squares) for the cross-core sum
                obuf = ypool.tile([128, PAIRS * BT], F16)
                agin = spool.tile([128, 2], F32)
                nc.vector.tensor_reduce(
                    agin[:, 0:1],
                    sumc[:],
                    axis=mybir.AxisListType.X,
                    op=mybir.AluOpType.add,
                )
                nc.scalar.activation(
                    obuf[:], ysb[:], AF.Square, accum_out=agin[:, 1:2]
                )

                agi = dram.tile([128, 2], F32)
                ago = dram.tile([128, 2], F32)
                nc.sync.dma_start(agi[:], agin[:])
                if timeline:
                    nc.sync.dma_start(ago[:], agi[:])
                else:
                    nc.gpsimd.collective_compute(
                        "AllReduce",
                        mybir.AluOpType.add,
                        replica_groups=[list(range(N_CORES))],
                        ins=[agi.opt()],
                        outs=[ago.opt()],
                    )
                # load both partition halves (lp=0/1) side by side on
                # partitions 0-63, so no cross-half shuffle is needed
                g = spool.tile([128, 4], F32)
                nc.sync.dma_start(
                    g[0:64, :].rearrange("p (h v) -> p h v", h=2),
                    ago.rearrange("(h p) v -> p h v", h=2),
                )
                s2 = spool.tile([128, 2], F32)
                nc.vector.tensor_add(s2[0:64, :], g[0:64, 0:2], g[0:64, 2:4])
                mm2 = spool.tile([128, 2], F32)
                nc.scalar.mul(
                    mm2[0:64, :], s2[0:64, :], 1.0 / (2 * N_CORES * PAIRS * BT)
                )

                # scale = gamma * rsqrt(var+eps); shift = beta - mean*scale
                var = spool.tile([128, 1], F32)
                nc.vector.tensor_mul(var[0:64], mm2[0:64, 0:1], mm2[0:64, 0:1])
                nc.vector.tensor_sub(var[0:64], mm2[0:64, 1:2], var[0:64])
                sdt = spool.tile([128, 1], F32)
                nc.scalar.activation(
                    sdt[0:64], var[0:64], AF.Sqrt, bias=ppt[0:64, 11:12]
                )
                st = spool.tile([128, 1], F32)
                nc.vector.reciprocal(st[0:64], sdt[0:64])
                sclsht = spool.tile([128, 2], F32)
                nc.vector.tensor_mul(sclsht[0:64, 0:1], st[0:64], ppt[0:64, 8:9])
                nc.vector.tensor_mul(sclsht[0:64, 1:2], mm2[0:64, 0:1], sclsht[0:64, 0:1])
                nc.vector.tensor_sub(sclsht[0:64, 1:2], ppt[0:64, 9:10], sclsht[0:64, 1:2])
                nc.sync.dma_start(sclsht[64:128, :], sclsht[0:64, :])

                # fused BN affine + PReLU, split in quarters so each
                # slice's output DMA overlaps the next slice's activation
                yov = yo.rearrange("(j lp) o n -> (lp o) j n", lp=2)
                NSP = 4
                HB = PAIRS * BT // NSP
                JS = PAIRS // NSP
                for h in range(NSP):
                    nc.scalar.activation(
                        obuf[:, h * HB : (h + 1) * HB],
                        ysb[:, h * HB : (h + 1) * HB],
                        AF.Prelu if prelu else AF.Identity,
                        bias=sclsht[:, 1:2],
                        scale=sclsht[:, 0:1],
                        alpha=ppt[:, 10:11],
                    )
                    nc.sync.dma_start(
                        yov[:, h * JS : (h + 1) * JS, :],
                        obuf[:, h * HB : (h + 1) * HB].rearrange(
                            "p (j n) -> p j n", n=BT
                        ),
                    )
    nc.compile()
    return nc


def _get_nc():
    if "nc" not in _CACHE:
        _CACHE["nc"] = _build_nc()
    return _CACHE["nc"]


def _prep_in_maps(x, weight, bias, gamma, beta, prelu_a):
    x = np.ascontiguousarray(x, dtype=np.float32)
    weight = np.asarray(weight, dtype=np.float32)
    bias = np.asarray(bias, dtype=np.float32)
    gamma = np.asarray(gamma, dtype=np.float32)
    beta = np.asarray(beta, dtype=np.float32)
    prelu_a = np.float32(np.asarray(prelu_a))

    # padded tap-row-major input: xtp[j] = x[:, :, j-1, :] as [C, B*T], fp16
    xtp = np.zeros((H + 2, C, B, T), np.float16)
    xtp[1 : H + 1] = np.transpose(x, (2, 1, 0, 3))
    xtp = xtp.reshape(H + 2, C, BT)

    wv = weight.reshape(C, 3, O, L)  # [c, kh, o, l]
    lidx = np.arange(L).reshape(N_CORES, PAIRS, 2)
    lA, lB = lidx[:, :, 0], lidx[:, :, 1]

    def pick(kh, l2):  # -> [core, j, c, o]
        return np.transpose(wv[:, kh][:, :, l2], (2, 3, 0, 1))

    wball = np.zeros((N_CORES, PAIRS, 3, 2, C, 2, O), np.float32)
    wball[:, :, 0, 0, :, 0, :] = pick(0, lA)
    wball[:, :, 0, 1, :, 0, :] = pick(1, lA)
    wball[:, :, 1, 0, :, 0, :] = pick(2, lA)
    wball[:, :, 1, 0, :, 1, :] = pick(0, lB)
    wball[:, :, 1, 1, :, 1, :] = pick(1, lB)
    wball[:, :, 2, 0, :, 1, :] = pick(2, lB)
    # [core, mm=3*PAIRS, K=128, M=128] -> partition-major [core, K, mm, M]
    wball = wball.reshape(N_CORES, 3 * PAIRS, 128, 128)
    wball = np.ascontiguousarray(
        np.transpose(wball, (0, 2, 1, 3)), dtype=np.float16
    ).reshape(N_CORES, 128, 3 * PAIRS * 128)

    bv = bias.reshape(O, N_CORES, PAIRS, 2)  # [o, core, j, lp]
    cball = np.ascontiguousarray(
        np.transpose(bv, (1, 3, 0, 2)).reshape(N_CORES, 128, PAIRS)
    )

    in_maps = []
    for i in range(N_CORES):
        slab = xtp[32 * i : 32 * i + SLAB]                # [34, C, BT]
        slab = slab.reshape(NT, 2, C, BT)                 # [t, jpar, c, n]
        slab = np.ascontiguousarray(np.transpose(slab, (1, 2, 0, 3)))
        pp = np.zeros((128, 12), np.float32)
        pp[:, :PAIRS] = cball[i]
        pp[:, 8] = np.concatenate([gamma, gamma])
        pp[:, 9] = np.concatenate([beta, beta])
        pp[:, 10] = prelu_a
        pp[:, 11] = BN_EPS
        in_maps.append(
            {
                "xs": slab.reshape(128, NT * BT),
                "wb": wball[i],
                "pp": pp,
            }
        )
    return in_maps


def _unshard(results):
    outs = [
        results[i]["yo"].reshape(LC, O, B, T).transpose(2, 1, 0, 3)
        for i in range(N_CORES)
    ]
    return np.ascontiguousarray(
        np.concatenate(outs, axis=2), dtype=np.float32
    )


def kernel(x, weight, bias, gamma, beta, prelu_a):
    nc = _get_nc()
    in_maps = _prep_in_maps(x, weight, bias, gamma, beta, prelu_a)
    res = bass_utils.run_bass_kernel_spmd(
        nc, in_maps, core_ids=list(range(N_CORES)), trace=False
    )
    return _unshard(res.results)


# revision 28
# speedup vs baseline: 1.7487x; 1.0679x over previous
"""Trainium2 Bass kernel for nn_LocalDenseConv1D (unfold conv + BN(train) + PReLU).

Strategy: shard the 128 output positions (L) across 8 NeuronCores (16 each).
Host pre-transposes x [B,C,H,T] -> per-core fp16 slab [128, 17, 2048] whose
partition dim is (row-parity, channel) so one contiguous DMA stages the whole
input. The locally-connected contraction runs as 96 fp16 matmuls per core
(K=128 = 2 tap rows x 64 channels, M=128 = 2 output positions x 64
out-channels, N=512 (b,t) columns). Each output-position pair accumulates its
4 BT-chunks into 4 PSUM banks of one [128, 2048] tile, so a single ScalarE
activation evicts the pair (adding the conv bias) while its accumulator
emits the pair's running sum; one ScalarE Square pass emits sum-of-squares.
A tiny AllReduce sums per-channel (sum, sum-sq) across cores, and a single
fused ScalarE Prelu applies the BN affine + PReLU over the whole
[128, 16384] block before one strided DMA writes the fp16 output (upcast to
fp32 on host).
"""
import numpy as np

import concourse.bass as bass
import concourse.tile as tile
from concourse import bacc, mybir
from concourse import bass_utils

F32 = mybir.dt.float32
F16 = mybir.dt.float16
AF = mybir.ActivationFunctionType

N_CORES = 8
B, C, H, T = 8, 64, 256, 256
O, L = 64, 128
BT = B * T                  # 2048 moving columns total
LC = L // N_CORES           # 16 output positions per core
PAIRS = LC // 2             # 8 pairs
SLAB = 2 * LC + 2           # 34 tap rows per core
NT = SLAB // 2              # 17 tap-pair tiles
CW = 512                    # matmul moving-dim chunk (PSUM bank)
NCH = BT // CW              # 4 chunks
BN_EPS = 1e-5

_CACHE = {}


def _build_nc(reps=1, timeline=False, prelu=True):
    nc = bacc.Bacc(
        "TRN2",
        target_bir_lowering=False,
        debug=False,
        enable_asserts=True,
        num_devices=1 if timeline else N_CORES,
    )
    xs = nc.dram_tensor("xs", [128, NT * BT], F16, kind="ExternalInput").ap()
    wb = nc.dram_tensor("wb", [128, 3 * PAIRS * 128], F16, kind="ExternalInput").ap()
    pp = nc.dram_tensor("pp", [128, 12], F32, kind="ExternalInput").ap()
    yo = nc.dram_tensor("yo", [LC, O, BT], F16, kind="ExternalOutput").ap()

    with tile.TileContext(nc) as tc:
        with (
            tc.tile_pool(name="xc", bufs=1) as xpool,
            tc.tile_pool(name="wp", bufs=1) as wpool,
            tc.tile_pool(name="yp", bufs=1) as ypool,
            tc.tile_pool(name="sp", bufs=1) as spool,
            tc.tile_pool(name="ps", bufs=2, space="PSUM") as psum,
            tc.tile_pool(name="dr", bufs=1, space="DRAM") as dram,
        ):
            for _rep in range(reps):
                wt = wpool.tile([128, 3 * PAIRS * 128], F16)
                nc.sync.dma_start(wt[:], wb[:])
                ppt = spool.tile([128, 12], F32)
                nc.sync.dma_start(ppt[:], pp[:])
                # stage the input slab one tap-tile (512KB) per DMA: pair j
                # only waits for tiles 2j..2j+2, so matmuls overlap the
                # remaining input DMA maximally
                xt = xpool.tile([128, NT * BT], F16)
                for tti in range(NT):
                    nc.sync.dma_start(
                        xt[:, tti * BT : (tti + 1) * BT],
                        xs[:, tti * BT : (tti + 1) * BT],
                    )

                ysb = ypool.tile([128, PAIRS * BT], F32)
                sumc = spool.tile([128, PAIRS], F32)
                for j in range(PAIRS):
                    pt = psum.tile([128, NCH * CW], F32, tag="acc")
                    for cc in range(NCH):
                        for k in range(3):
                            mm = j * 3 + k
                            nc.tensor.matmul(
                                pt[:, cc * CW : (cc + 1) * CW],
                                lhsT=wt[:, mm * 128 : (mm + 1) * 128],
                                rhs=xt[
                                    :,
                                    (2 * j + k) * BT + cc * CW :
                                    (2 * j + k) * BT + (cc + 1) * CW,
                                ],
                                start=(k == 0),
                                stop=(k == 2),
                            )
                    # evict whole pair (4 banks) + conv bias in one activation;
                    # the ScalarE accumulator gives this pair's sum for free
                    nc.scalar.activation(
                        ysb[:, j * BT : (j + 1) * BT],
                        pt[:],
                        AF.Identity,
                        bias=ppt[:, j : j + 1],
                        scale=1.0,
                        accum_out=sumc[:, j : j + 1],
                    )

                # per-partition (sum, sum-of-squares) for the cross-core sum;
                # the Square pass runs in quarters so most of it hides under
                # the matmul/DMA window instead of gating the collective
                obuf = ypool.tile([128, PAIRS * BT], F16)
                agin = spool.tile([128, 2], F32)
                nc.vector.tensor_reduce(
                    agin[:, 0:1],
                    sumc[:],
                    axis=mybir.AxisListType.X,
                    op=mybir.AluOpType.add,
                )
                SQH = PAIRS * BT // 4
                sqacc = spool.tile([128, 4], F32)
                for h in range(4):
                    nc.scalar.activation(
                        obuf[:, h * SQH : (h + 1) * SQH],
                        ysb[:, h * SQH : (h + 1) * SQH],
                        AF.Square,
                        accum_out=sqacc[:, h : h + 1],
                    )
                nc.vector.tensor_reduce(
                    agin[:, 1:2],
                    sqacc[:],
                    axis=mybir.AxisListType.X,
                    op=mybir.AluOpType.add,
                )

                agi = dram.tile([128, 2], F32)
                ago = dram.tile([128, 2], F32)
                nc.sync.dma_start(agi[:], agin[:])
                if timeline:
                    nc.sync.dma_start(ago[:], agi[:])
                else:
                    nc.gpsimd.collective_compute(
                        "AllReduce",
                        mybir.AluOpType.add,
                        replica_groups=[list(range(N_CORES))],
                        ins=[agi.opt()],
                        outs=[ago.opt()],
                    )
                # load the (lp=0/1) halves side by side onto BOTH partition
                # halves, so the whole fixup computes once on 128 partitions
                # and no final cross-half broadcast is needed
                g = spool.tile([128, 4], F32)
                for base in (0, 64):
                    nc.sync.dma_start(
                        g[base : base + 64, :].rearrange(
                            "p (h v) -> p h v", h=2
                        ),
                        ago.rearrange("(h p) v -> p h v", h=2),
                    )
                s2 = spool.tile([128, 2], F32)
                nc.vector.tensor_add(s2[:], g[:, 0:2], g[:, 2:4])
                mm2 = spool.tile([128, 2], F32)
                nc.scalar.mul(
                    mm2[:], s2[:], 1.0 / (2 * N_CORES * PAIRS * BT)
                )

                # scale = gamma * rsqrt(var+eps); shift = beta - mean*scale
                var = spool.tile([128, 1], F32)
                nc.vector.tensor_mul(var[:], mm2[:, 0:1], mm2[:, 0:1])
                nc.vector.tensor_sub(var[:], mm2[:, 1:2], var[:])
                sdt = spool.tile([128, 1], F32)
                nc.scalar.activation(
                    sdt[:], var[:], AF.Sqrt, bias=ppt[:, 11:12]
                )
                st = spool.tile([128, 1], F32)
                nc.vector.reciprocal(st[:], sdt[:])
                sclsht = spool.tile([128, 2], F32)
                nc.vector.tensor_mul(sclsht[:, 0:1], st[:], ppt[:, 8:9])
                nc.vector.tensor_mul(sclsht[:, 1:2], mm2[:, 0:1], sclsht[:, 0:1])
                nc.vector.tensor_sub(sclsht[:, 1:2], ppt[:, 9:10], sclsht[:, 1:2])

                # fused BN affine + PReLU, split in quarters so each
                # slice's output DMA overlaps the next slice's activation
                yov = yo.rearrange("(j lp) o n -> (lp o) j n", lp=2)
                NSP = 8
                HB = PAIRS * BT // NSP
                JS = PAIRS // NSP
                for h in range(NSP):
                    nc.scalar.activation(
                        obuf[:, h * HB : (h + 1) * HB],
                        ysb[:, h * HB : (h + 1) * HB],
                        AF.Prelu if prelu else AF.Identity,
                        bias=sclsht[:, 1:2],
                        scale=sclsht[:, 0:1],
                        alpha=ppt[:, 10:11],
                    )
                    nc.sync.dma_start(
                        yov[:, h * JS : (h + 1) * JS, :],
                        obuf[:, h * HB : (h + 1) * HB].rearrange(
                            "p (j n) -> p j n", n=BT
                        ),
                    )
    nc.compile()
    return nc


def _get_nc():
    if "nc" not in _CACHE:
        _CACHE["nc"] = _build_nc()
    return _CACHE["nc"]


def _prep_in_maps(x, weight, bias, gamma, beta, prelu_a):
    x = np.ascontiguousarray(x, dtype=np.float32)
    weight = np.asarray(weight, dtype=np.float32)
    bias = np.asarray(bias, dtype=np.float32)
    gamma = np.asarray(gamma, dtype=np.float32)
    beta = np.asarray(beta, dtype=np.float32)
    prelu_a = np.float32(np.asarray(prelu_a))

    # padded tap-row-major input: xtp[j] = x[:, :, j-1, :] as [C, B*T], fp16
    xtp = np.zeros((H + 2, C, B, T), np.float16)
    xtp[1 : H + 1] = np.transpose(x, (2, 1, 0, 3))
    xtp = xtp.reshape(H + 2, C, BT)

    wv = weight.reshape(C, 3, O, L)  # [c, kh, o, l]
    lidx = np.arange(L).reshape(N_CORES, PAIRS, 2)
    lA, lB = lidx[:, :, 0], lidx[:, :, 1]

    def pick(kh, l2):  # -> [core, j, c, o]
        return np.transpose(wv[:, kh][:, :, l2], (2, 3, 0, 1))

    wball = np.zeros((N_CORES, PAIRS, 3, 2, C, 2, O), np.float32)
    wball[:, :, 0, 0, :, 0, :] = pick(0, lA)
    wball[:, :, 0, 1, :, 0, :] = pick(1, lA)
    wball[:, :, 1, 0, :, 0, :] = pick(2, lA)
    wball[:, :, 1, 0, :, 1, :] = pick(0, lB)
    wball[:, :, 1, 1, :, 1, :] = pick(1, lB)
    wball[:, :, 2, 0, :, 1, :] = pick(2, lB)
    # [core, mm=3*PAIRS, K=128, M=128] -> partition-major [core, K, mm, M]
    wball = wball.reshape(N_CORES, 3 * PAIRS, 128, 128)
    wball = np.ascontiguousarray(
        np.transpose(wball, (0, 2, 1, 3)), dtype=np.float16
    ).reshape(N_CORES, 128, 3 * PAIRS * 128)

    bv = bias.reshape(O, N_CORES, PAIRS, 2)  # [o, core, j, lp]
    cball = np.ascontiguousarray(
        np.transpose(bv, (1, 3, 0, 2)).reshape(N_CORES, 128, PAIRS)
    )

    in_maps = []
    for i in range(N_CORES):
        slab = xtp[32 * i : 32 * i + SLAB]                # [34, C, BT]
        slab = slab.reshape(NT, 2, C, BT)                 # [t, jpar, c, n]
        slab = np.ascontiguousarray(np.transpose(slab, (1, 2, 0, 3)))
        pp = np.zeros((128, 12), np.float32)
        pp[:, :PAIRS] = cball[i]
        pp[:, 8] = np.concatenate([gamma, gamma])
        pp[:, 9] = np.concatenate([beta, beta])
        pp[:, 10] = prelu_a
        pp[:, 11] = BN_EPS
        in_maps.append(
            {
                "xs": slab.reshape(128, NT * BT),
                "wb": wball[i],
                "pp": pp,
            }
        )
    return in_maps


def _unshard(results):
    outs = [
        results[i]["yo"].reshape(LC, O, B, T).transpose(2, 1, 0, 3)
        for i in range(N_CORES)
    ]
    return np.ascontiguousarray(
        np.concatenate(outs, axis=2), dtype=np.float32
    )


def kernel(x, weight, bias, gamma, beta, prelu_a):
    nc = _get_nc()
    in_maps = _prep_in_maps(x, weight, bias, gamma, beta, prelu_a)
    res = bass_utils.run_bass_kernel_spmd(
        nc, in_maps, core_ids=list(range(N_CORES)), trace=False
    )
    return _unshard(res.results)
